# revision 9
# baseline (speedup 1.0000x reference)
"""LogNCDE Trainium2 kernel — full-input contract.

Computes depth-2 log-signature windows of the input path, the log-ODE scan
h_{n+1} = h_n + s1·V(h_n) + s2·[V_i,V_j](h_n), and the linear readout, on
8 NeuronCores (pure batch data-parallelism: 4 of the 32 batch elements per
core).

The Lie bracket is evaluated analytically:

    sum_{i<j} s2_ij [V_i,V_j] = sum_{d,e} A[e,d] (1-V_d^2) ⊙ (G_d @ V_e)

with A = 0.5(M - M^T) the Levy-area matrix and G = du/dh the linearized
MLP (gated by sigmoid of the forward pre-activations), i.e. one batched
JVP through the vf MLP per step followed by an elementwise multiply with a
per-step coefficient row and a grouped reduction.

This walrus build cannot compile Softplus/Sigmoid/Tanh together (no common
ACT table set; Softplus crashes lower_act outright), so everything is built
from the `natural_log_exp_and_others` set only:

    softplus(a) = ln(exp(a) + 1)           (+1 folded into the ACT bias)
    sigmoid(a)  = 1 - R,  R = recip(exp(a)+1)      (DVE reciprocal)
    tanh(u)     = 1 - 2*recip(exp(2u)+1)

The affine parts of sigmoid/tanh are folded into pre-negated/scaled weight
copies (Wv0·(-2), -Wv1, sign-flipped Levy coefficients), so the JVP still
costs one gate multiply per layer.

Scan state h lives on partitions (64p = state dim); the scan is fully
unrolled (128 steps, ~35 instructions each).  Per-step coefficient rows
(Levy areas, increments) are computed on device with the 128 windows on
partitions, then replicated across the 64 state partitions via a DRAM
round-trip so the tail multiply can read them with a stride-0 AP.
"""

import sys
import numpy as np

sys.path.insert(0, "/opt/trn_rl_repo")

import concourse.bass as bass
import concourse.mybir as mybir
from concourse import tile
from concourse.bass_utils import run_bass_kernel_spmd

D = 8
S = 64
H = 128
OUT = 8
T = 2049
B = 32
WIN = 16
NWIN = (T - 1) // WIN  # 128
NC_CORES = 8
BC = B // NC_CORES  # 4 batch elements per core

F32 = mybir.dt.float32
AF = mybir.ActivationFunctionType
OP = mybir.AluOpType
X = mybir.AxisListType.X

_PATCHED = False


def _patch_tile_drain():
    """This walrus build rejects sem waits attached to Drain instructions
    ("Too many sync wait commands").  Emit the end-of-kernel waits as
    standalone wait_ge instructions instead."""
    global _PATCHED
    if _PATCHED:
        return
    _PATCHED = True

    def _drain_and_barrier(self, tick_clock, wait_clock):
        nc = self.nc
        carrier = nc.sync.nop()
        wait_clock.add_sem_waits(
            carrier.ins, tile.ScopedClock({None: tick_clock.global_clock})
        )
        ws = list(carrier.ins.sync_info.on_wait)
        carrier.ins.sync_info = mybir.SyncInfo(on_wait=[], on_update=[])
        by_name = {s.name: s for s in self.sems.allocated().values()}
        for w in ws:
            s = by_name.get(w.ant_name)
            if s is not None:
                nc.sync.wait_ge(s, w.wait_value)
        nc.sync.drain()
        nc.all_engine_barrier()
        popped = nc._tile_sem_poison_stack.pop()
        assert popped is self._sem_poison
        nc.clear_and_free_semaphores(list(self.sems.allocated().values()))
        nc.all_engine_barrier()

    tile.TileContext._drain_and_barrier = _drain_and_barrier


def _ap(base, offset, dims):
    return bass.AP(tensor=base.tensor, offset=offset, ap=[list(d) for d in dims])


WEIGHT_SPECS = [
    ("Wi0T", [D, H]), ("BI0", [1, H]),
    ("Wi1T", [H, H]), ("BI1", [1, H]),
    ("Wi2T", [H, S]), ("BI2", [1, S]),
    ("Wv0T", [S, H]), ("BV0", [1, H]),
    ("Wv1T", [H, H]), ("BV1", [1, H]),
    ("Wv2T", [H, D * S]),
    ("BV2", [D, S]), ("EYE", [D, 4 * D]),
    ("Wv0Tm2", [S, H]),   # -2 * Wv0^T   (tanh affine folded into JVP layer 0)
    ("RS0", [1, H]),      # row sums of Wv0 (Wv0 @ ones)
    ("ONES", [1, 512]),
    ("Wv1Tn", [H, H]),    # -Wv1^T       (gate sign fold)
    ("WrT", [S, OUT]), ("BR", [1, OUT]),
]



def _hoist_excess_waits(nc, max_waits=1):
    """walrus (this build) supports at most one sync-wait on most
    instructions and none on Drain; hoist the extras into standalone
    EventSemaphore instructions just before the owner, same engine."""
    cnt = 0
    for f in nc.m.functions:
        for bb in f.blocks:
            insts = list(bb.instructions)
            out = []
            changed = False
            for ins in insts:
                si = ins.sync_info
                ws = list(si.on_wait) if si is not None else []
                limit = 0 if type(ins).__name__ == "InstDrain" else max_waits
                if len(ws) > limit:
                    keep, extra = ws[:limit], ws[limit:]
                    for wi in extra:
                        cnt += 1
                        ev = mybir.InstEventSemaphore(name=f"HOISTW-{cnt}", ins=[], outs=[])
                        ev.engine = ins.engine
                        ev.sync_info = mybir.SyncInfo(on_wait=[wi], on_update=[])
                        out.append(ev)
                    ins.sync_info = mybir.SyncInfo(on_wait=keep, on_update=list(si.on_update))
                    changed = True
                out.append(ins)
            if changed:
                bb.instructions = out
    return cnt


def build_nc():
    _patch_tile_drain()
    nc = bass.Bass()

    xd = nc.dram_tensor("x", [BC, T, D], F32, kind="ExternalInput")
    wi = {}
    for name, shape in WEIGHT_SPECS:
        wi[name] = nc.dram_tensor(name, shape, F32, kind="ExternalInput")
    outd = nc.dram_tensor("out", [BC, NWIN + 1, OUT], F32, kind="ExternalOutput")

    with tile.TileContext(nc) as tc:
        with (
            tc.tile_pool(name="singles", bufs=1) as SG,
            tc.tile_pool(name="dram", bufs=1, space="DRAM") as DR,
        ):
            w = {}
            for name, shape in WEIGHT_SPECS:
                w[name] = SG.tile(list(shape), F32, tag=name, name=name)
                nc.sync.dma_start(out=w[name][:], in_=wi[name][:])

            hist = SG.tile([S, 4 * (NWIN + 1)], F32, tag="hist")  # (64, 516)
            s1full = SG.tile([S, NWIN * 32], F32, tag="s1full")  # (64, 4096)
            afull = [SG.tile([S, 32 * 256], F32, tag=f"af{c}", name=f"af{c}")
                     for c in range(4)]
            a_dram = DR.tile([NWIN, 256], F32)
            s1_dram = DR.tile([NWIN, 32], F32)
            atile = SG.tile([NWIN, 256], F32, tag="atile")
            s1sb = SG.tile([NWIN, 32], F32, tag="s1sb")

            xbase = xd[:]

            # ---------------- phase A: log-signatures ----------------
            with (
                tc.tile_pool(name="pa", bufs=2) as PA,
                tc.tile_pool(name="pap", bufs=2, space="PSUM") as PAP,
            ):
                for b in range(BC):
                    xa = PA.tile([128, 128], F32, tag="xa")
                    nc.sync.dma_start(
                        out=xa[:], in_=_ap(xbase, b * T * D, [[128, 128], [1, 128]])
                    )
                    xw = PA.tile([128, 128], F32, tag="xw")
                    nc.sync.dma_start(
                        out=xw[:], in_=_ap(xbase, b * T * D + D, [[128, 128], [1, 128]])
                    )
                    # deltas, with 64 zero pad columns in front for the shifts
                    dlt = PA.tile([128, 192], F32, tag="dlt")
                    nc.vector.memset(dlt[:, 0:64], 0.0)
                    nc.vector.tensor_sub(dlt[:, 64:192], xw[:], xa[:])
                    dview = dlt[:, 64:192]
                    # s1[n, e] (scaled by -2 for the tail STT) -> s1sb col e*4+b
                    s1r = PA.tile([128, 8], F32, tag="s1r")
                    nc.vector.tensor_reduce(
                        out=s1r[:],
                        in_=dview.rearrange("p (w d) -> p d w", d=D),
                        axis=X,
                        op=OP.add,
                    )
                    nc.vector.tensor_scalar(
                        out=_ap(s1sb[:], b, [[32, 128], [4, 8]]),
                        in0=s1r[:], scalar1=-2.0, scalar2=None, op0=OP.mult,
                    )
                    # inclusive prefix sums over w (shifts of 1,2,4,8 windows)
                    cprev = dlt
                    for k, sh in enumerate((8, 16, 32, 64)):
                        cn = PA.tile([128, 192], F32, tag=f"c{k}")
                        nc.vector.memset(cn[:, 0:64], 0.0)
                        nc.vector.tensor_add(
                            cn[:, 64:192], cprev[:, 64:192], cprev[:, 64 - sh:192 - sh]
                        )
                        cprev = cn
                    ex = PA.tile([128, 128], F32, tag="ex")
                    nc.vector.tensor_sub(ex[:], cprev[:, 64:192], dlt[:, 64:192])
                    # M[n, i, j] = sum_w ex[w, i] * dlt[w, j]
                    prod = PA.tile([128, 1024], F32, tag="prod")
                    exb = ex[:]
                    nc.vector.tensor_mul(
                        prod[:],
                        _ap(exb, exb.offset, [list(exb.ap[0]), [1, 8], [0, 8], [8, 16]]),
                        _ap(dview, dview.offset,
                            [list(dview.ap[0]), [0, 8], [1, 8], [8, 16]]),
                    )
                    mred = PA.tile([128, 64], F32, tag="mred")
                    nc.vector.tensor_reduce(
                        out=mred[:],
                        in_=prod[:].rearrange("p (i j w) -> p i j w", i=8, j=8),
                        axis=X,
                        op=OP.add,
                    )
                    # coefficient A4[n, d, e] = 2*(M[e,d] - M[d,e])
                    ldf = PA.tile([128, 64], F32, tag="ldf")
                    mb = mred[:]
                    nc.vector.tensor_sub(
                        ldf[:],
                        _ap(mb, mb.offset, [list(mb.ap[0]), [1, 8], [8, 8]]),
                        _ap(mb, mb.offset, [list(mb.ap[0]), [8, 8], [1, 8]]),
                    )
                    nc.vector.tensor_scalar(
                        out=_ap(atile[:], b, [[256, 128], [32, 8], [4, 8]]),
                        in0=ldf[:], scalar1=2.0, scalar2=None, op0=OP.mult,
                    )

                nc.sync.dma_start(out=a_dram[:], in_=atile[:])
                nc.sync.dma_start(out=s1_dram[:], in_=s1sb[:])
                # replicate across the 64 state partitions via DRAM round-trip
                adb = a_dram[:]
                for c in range(4):
                    nc.gpsimd.dma_start(
                        out=afull[c][:],
                        in_=_ap(adb, c * 32 * 256, [[0, S], [1, 32 * 256]]),
                    )
                s1db = s1_dram[:]
                nc.gpsimd.dma_start(
                    out=s1full[:], in_=_ap(s1db, 0, [[0, S], [1, NWIN * 32]])
                )

                # ---------------- initial MLP -> hist[:, 0:4] ----------------
                x0 = PA.tile([D, BC], F32, tag="x0")
                nc.sync.dma_start(out=x0[:], in_=_ap(xbase, 0, [[1, D], [T * D, BC]]))
                pi0 = PAP.tile([H, BC], F32, tag="pi")
                nc.tensor.matmul(pi0[:], w["BI0"][:], w["ONES"][:, 0:BC], start=True, stop=False)
                nc.tensor.matmul(pi0[:], w["Wi0T"][:], x0[:], start=False, stop=True)
                ei0 = PA.tile([H, BC], F32, tag="ei0")
                nc.scalar.activation(ei0[:], pi0[:], AF.Exp)
                zi0 = PA.tile([H, BC], F32, tag="zi0")
                nc.scalar.activation(zi0[:], ei0[:], AF.Ln, bias=1.0)
                pi1 = PAP.tile([H, BC], F32, tag="pi")
                nc.tensor.matmul(pi1[:], w["BI1"][:], w["ONES"][:, 0:BC], start=True, stop=False)
                nc.tensor.matmul(pi1[:], w["Wi1T"][:], zi0[:], start=False, stop=True)
                ei1 = PA.tile([H, BC], F32, tag="ei1")
                nc.scalar.activation(ei1[:], pi1[:], AF.Exp)
                zi1 = PA.tile([H, BC], F32, tag="zi1")
                nc.scalar.activation(zi1[:], ei1[:], AF.Ln, bias=1.0)
                ph = PAP.tile([S, BC], F32, tag="ph")
                nc.tensor.matmul(ph[:], w["BI2"][:], w["ONES"][:, 0:BC], start=True, stop=False)
                nc.tensor.matmul(ph[:], w["Wi2T"][:], zi1[:], start=False, stop=True)
                nc.scalar.activation(hist[:, 0:BC], ph[:], AF.Copy)

            # ---------------- phase B: the scan ----------------
            with (
                tc.tile_pool(name="psa", bufs=2, space="PSUM") as PSA,
                tc.tile_pool(name="psu", bufs=2, space="PSUM") as PSU,
                tc.tile_pool(name="pst", bufs=2, space="PSUM") as PST,
                tc.tile_pool(name="psw", bufs=2, space="PSUM") as PSW,
                tc.tile_pool(name="step", bufs=2) as ST,
            ):
                for n in range(NWIN):
                    hcur = hist[:, 4 * n:4 * n + 4]
                    # ---- forward MLP (softplus via exp/ln) ----
                    a0 = PSA.tile([H, BC], F32, tag="psa")
                    nc.tensor.matmul(a0[:], w["BV0"][:], w["ONES"][:, 0:BC], start=True, stop=False)
                    nc.tensor.matmul(a0[:], w["Wv0T"][:], hcur, start=False, stop=True)
                    e0 = ST.tile([H, BC], F32, tag="e0")
                    nc.scalar.activation(e0[:], a0[:], AF.Exp)
                    z0 = ST.tile([H, BC], F32, tag="z0")
                    nc.scalar.activation(z0[:], e0[:], AF.Ln, bias=1.0)
                    a1 = PSA.tile([H, BC], F32, tag="psa")
                    nc.tensor.matmul(a1[:], w["BV1"][:], w["ONES"][:, 0:BC], start=True, stop=False)
                    nc.tensor.matmul(a1[:], w["Wv1T"][:], z0[:], start=False, stop=True)
                    e1 = ST.tile([H, BC], F32, tag="e1")
                    nc.scalar.activation(e1[:], a1[:], AF.Exp)
                    z1 = ST.tile([H, BC], F32, tag="z1")
                    nc.scalar.activation(z1[:], e1[:], AF.Ln, bias=1.0)
                    # gates R = 1/(1+E)  (sigma = 1-R, signs folded downstream)
                    e0p = ST.tile([H, BC], F32, tag="e0p")
                    nc.vector.tensor_scalar(
                        out=e0p[:], in0=e0[:], scalar1=1.0, scalar2=None, op0=OP.add
                    )
                    r0 = ST.tile([H, BC], F32, tag="r0")
                    nc.vector.reciprocal(r0[:], e0p[:])
                    e1p = ST.tile([H, BC], F32, tag="e1p")
                    nc.vector.tensor_scalar(
                        out=e1p[:], in0=e1[:], scalar1=1.0, scalar2=None, op0=OP.add
                    )
                    r1 = ST.tile([H, BC], F32, tag="r1")
                    nc.vector.reciprocal(r1[:], e1p[:])
                    # ---- u layer: psum = Wv2 z1 + bv2, tiled (64p=a, e*4+b) ----
                    U = PSU.tile([S, 4 * D], F32, tag="psu")
                    nc.tensor.matmul(U[:], w["BV2"][:], w["EYE"][:], start=True, stop=False)
                    for e in range(D):
                        nc.tensor.matmul(
                            U[:, 4 * e:4 * e + 4],
                            w["Wv2T"][:, S * e:S * e + S],
                            z1[:], start=False, stop=True,
                        )
                    # Re = recip(exp(2u)+1);  V = 1 - 2 Re (folds into weights)
                    ee = ST.tile([S, 4 * D], F32, tag="ee")
                    nc.scalar.activation(ee[:], U[:], AF.Exp, scale=2.0)
                    eep = ST.tile([S, 4 * D], F32, tag="eep")
                    nc.vector.tensor_scalar(
                        out=eep[:], in0=ee[:], scalar1=1.0, scalar2=None, op0=OP.add
                    )
                    re = ST.tile([S, 4 * D], F32, tag="re")
                    nc.vector.reciprocal(re[:], eep[:])

                    # off-path pieces of the update tile
                    UPD = ST.tile([S, 292], F32, tag="upd")
                    nc.vector.tensor_copy(UPD[:, 288:292], hcur)
                    # ssm = (Re-1)*Re ; M2 = ssm ⊙ A4row  (tail coefficients)
                    ssm = ST.tile([S, 4 * D], F32, tag="ssm")
                    nc.vector.scalar_tensor_tensor(
                        out=ssm[:], in0=re[:], scalar=1.0, in1=re[:],
                        op0=OP.subtract, op1=OP.mult,
                    )
                    # s1 part: (Re - 0.5) * (-2 s1row)
                    nc.vector.scalar_tensor_tensor(
                        out=UPD[:, 256:288], in0=re[:], scalar=0.5,
                        in1=s1full[:, 32 * n:32 * n + 32],
                        op0=OP.subtract, op1=OP.mult,
                    )

                    # ---- JVP ----
                    t0 = PST.tile([H, 4 * D], F32, tag="pst")
                    nc.tensor.matmul(t0[:], w["RS0"][:], w["ONES"][:, 0:4 * D], start=True, stop=False)
                    nc.tensor.matmul(t0[:], w["Wv0Tm2"][:], re[:], start=False, stop=True)
                    t0g = ST.tile([H, 4 * D], F32, tag="t0g")  # = -t0*g0
                    r0b = r0[:]
                    nc.vector.scalar_tensor_tensor(
                        out=t0g[:],
                        in0=_ap(r0b, r0b.offset, [list(r0b.ap[0]), [0, 8], [1, 4]]),
                        scalar=1.0, in1=t0[:], op0=OP.subtract, op1=OP.mult,
                    )
                    t1 = PST.tile([H, 4 * D], F32, tag="pst")
                    nc.tensor.matmul(t1[:], w["Wv1Tn"][:], t0g[:], start=True, stop=True)
                    t1g = ST.tile([H, 4 * D], F32, tag="t1g")  # = -t1*g1
                    r1b = r1[:]
                    nc.vector.scalar_tensor_tensor(
                        out=t1g[:],
                        in0=_ap(r1b, r1b.offset, [list(r1b.ap[0]), [0, 8], [1, 4]]),
                        scalar=1.0, in1=t1[:], op0=OP.subtract, op1=OP.mult,
                    )
                    M2 = ST.tile([S, 256], F32, tag="m2")
                    ssb = ssm[:]
                    ac = afull[n // 32]
                    nc.vector.tensor_mul(
                        M2[:],
                        _ap(ssb, ssb.offset, [list(ssb.ap[0]), [4, 8], [0, 8], [1, 4]]),
                        ac[:, 256 * (n % 32):256 * (n % 32) + 256],
                    )
                    W4 = PSW.tile([S, 256], F32, tag="psw")  # = -W4
                    for d in range(D):
                        nc.tensor.matmul(
                            W4[:, 32 * d:32 * d + 32],
                            w["Wv2T"][:, S * d:S * d + S],
                            t1g[:], start=True, stop=True,
                        )
                    nc.vector.tensor_mul(UPD[:, 0:256], W4[:], M2[:])
                    # h_next = grouped reduce over [bracket(64) | s1(8) | h(1)] groups
                    ub = UPD[:]
                    nc.vector.tensor_reduce(
                        out=hist[:, 4 * (n + 1):4 * (n + 1) + 4],
                        in_=_ap(ub, ub.offset, [list(ub.ap[0]), [1, 4], [4, 73]]),
                        axis=X,
                        op=OP.add,
                    )

            # ---------------- phase C: readout ----------------
            with (
                tc.tile_pool(name="psr", bufs=1, space="PSUM") as PSR,
                tc.tile_pool(name="ro", bufs=1) as RO,
            ):
                outsb = RO.tile([OUT, 4 * (NWIN + 1)], F32, tag="outsb")
                r0p = PSR.tile([OUT, 512], F32, tag="r0p")
                nc.tensor.matmul(r0p[:], w["BR"][:], w["ONES"][:], start=True, stop=False)
                nc.tensor.matmul(r0p[:], w["WrT"][:], hist[:, 0:512], start=False, stop=True)
                r1p = PSR.tile([OUT, 4], F32, tag="r1p")
                nc.tensor.matmul(r1p[:], w["BR"][:], w["ONES"][:, 0:4], start=True, stop=False)
                nc.tensor.matmul(r1p[:], w["WrT"][:], hist[:, 512:516], start=False, stop=True)
                nc.scalar.activation(outsb[:, 0:512], r0p[:], AF.Copy)
                nc.scalar.activation(outsb[:, 512:516], r1p[:], AF.Copy)
                ob = outsb[:]
                for b in range(BC):
                    nc.sync.dma_start(
                        out=_ap(outd[:], b * (NWIN + 1) * OUT, [[1, OUT], [OUT, NWIN + 1]]),
                        in_=_ap(ob, ob.offset + b, [list(ob.ap[0]), [4, NWIN + 1]]),
                    )
    _hoist_excess_waits(nc)
    return nc


def host_inputs(ts, x, Wi0, bi0, Wi1, bi1, Wi2, bi2,
                Wv0, bv0, Wv1, bv1, Wv2, bv2, Wr, br):
    """Per-core input maps (weights replicated, batch sharded)."""
    f = lambda a: np.ascontiguousarray(np.asarray(a, dtype=np.float32))
    eye = np.zeros((D, 4 * D), np.float32)
    for k in range(D):
        eye[k, 4 * k:4 * k + 4] = 1.0
    shared = {
        "Wi0T": f(Wi0.T), "BI0": f(bi0).reshape(1, H),
        "Wi1T": f(Wi1.T), "BI1": f(bi1).reshape(1, H),
        "Wi2T": f(Wi2.T), "BI2": f(bi2).reshape(1, S),
        "Wv0T": f(Wv0.T), "BV0": f(bv0).reshape(1, H),
        "Wv1T": f(Wv1.T), "BV1": f(bv1).reshape(1, H),
        "Wv2T": f(Wv2.T),
        "BV2": f(bv2).reshape(D, S),
        "EYE": eye,
        "Wv0Tm2": f(-2.0 * Wv0.T),
        "RS0": f(Wv0.sum(axis=1)).reshape(1, H),
        "ONES": np.ones((1, 512), np.float32),
        "Wv1Tn": f(-Wv1.T),
        "WrT": f(Wr.T), "BR": f(br).reshape(1, OUT),
    }
    x = f(x)
    in_maps = []
    for c in range(NC_CORES):
        m = dict(shared)
        m["x"] = np.ascontiguousarray(x[c * BC:(c + 1) * BC])
        in_maps.append(m)
    return in_maps


_CACHE = {}


def _make_runner(nc):
    """Persistent jitted PJRT runner (mirrors bass2jax.run_bass_via_pjrt's
    multi-core path, but keeps the jitted callable so repeat executions
    don't re-trace/re-compile)."""
    import jax
    from jax.experimental.shard_map import shard_map
    from jax.sharding import Mesh, PartitionSpec
    from concourse import bass2jax

    bass2jax.install_neuronx_cc_hook()
    partition_name = nc.partition_id_tensor.name if nc.partition_id_tensor else None
    in_names, out_names, out_avals, zero_shapes = [], [], [], []
    for alloc in nc.m.functions[0].allocations:
        if not isinstance(alloc, mybir.MemoryLocationSet):
            continue
        name = alloc.memorylocations[0].name
        if alloc.kind == "ExternalInput":
            if name != partition_name:
                in_names.append(name)
        elif alloc.kind == "ExternalOutput":
            out_names.append(name)
            shape = tuple(alloc.tensor_shape)
            dtype = mybir.dt.np(alloc.dtype)
            out_avals.append(jax.core.ShapedArray(shape, dtype))
            zero_shapes.append((shape, dtype))
    n_params, n_outs = len(in_names), len(out_names)
    all_in_names = tuple(in_names + out_names + ([partition_name] if partition_name else []))

    def _body(*args):
        operands = list(args)
        if partition_name:
            operands.append(bass2jax.partition_id_tensor())
        outs = bass2jax._bass_exec_p.bind(
            *operands,
            out_avals=tuple(out_avals),
            in_names=all_in_names,
            out_names=tuple(out_names),
            lowering_input_output_aliases=(),
            sim_require_finite=True,
            sim_require_nnan=True,
            nc=nc,
        )
        return tuple(outs)

    devices = jax.devices()[:NC_CORES]
    mesh = Mesh(np.asarray(devices), ("core",))
    sharded = jax.jit(
        shard_map(
            _body, mesh=mesh,
            in_specs=(PartitionSpec("core"),) * (n_params + n_outs),
            out_specs=(PartitionSpec("core"),) * n_outs,
            check_rep=False,
        ),
        donate_argnums=tuple(range(n_params, n_params + n_outs)),
        keep_unused=True,
    )

    def prep(in_maps):
        per_core = [[np.asarray(m[nm]) for nm in in_names] for m in in_maps]
        return [
            np.concatenate([per_core[c][i] for c in range(NC_CORES)], axis=0)
            for i in range(n_params)
        ]

    def run(concat_in):
        concat_zeros = [
            np.zeros((NC_CORES * s[0], *s[1:]), dt) for (s, dt) in zero_shapes
        ]
        out_arrs = sharded(*concat_in, *concat_zeros)
        jax.block_until_ready(out_arrs)
        return {out_names[i]: np.asarray(out_arrs[i]) for i in range(n_outs)}

    return prep, run


def _get_runner():
    if "runner" not in _CACHE:
        nc = build_nc()
        _CACHE["runner"] = _make_runner(nc)
    return _CACHE["runner"]


def kernel(**inputs) -> np.ndarray:
    in_maps = host_inputs(**inputs)
    prep, run = _get_runner()
    out = run(prep(in_maps))["out"]
    return np.ascontiguousarray(out.reshape(B, NWIN + 1, OUT).astype(np.float32))


if __name__ == "__main__":
    import os
    if not os.path.exists("/tmp/logncde_ref.npz"):
        import subprocess
        subprocess.run([sys.executable, "gen_expected.py"], check=True)
    dat = np.load("/tmp/logncde_ref.npz")
    inputs = {k: dat[k] for k in dat.files if k != "expected"}
    expected = dat["expected"]
    actual = kernel(**inputs)
    err = np.abs(actual - expected).max()
    print("max abs err:", err, "rel:", err / np.abs(expected).max())


# revision 28
# speedup vs baseline: 163.8198x; 163.8198x over previous
"""LogNCDE Trainium2 kernel — full-input contract.

Computes depth-2 log-signature windows of the input path, the log-ODE scan
h_{n+1} = h_n + s1·V(h_n) + s2·[V_i,V_j](h_n), and the linear readout, on
8 NeuronCores (pure batch data-parallelism: 4 of the 32 batch elements per
core).

The Lie bracket is evaluated analytically:

    sum_{i<j} s2_ij [V_i,V_j] = sum_{d,e} A[e,d] (1-V_d^2) ⊙ (G_d @ V_e)

with A = 0.5(M - M^T) the Levy-area matrix and G = du/dh the linearized
MLP (gated by sigmoid of the forward pre-activations), i.e. one batched
JVP through the vf MLP per step followed by an elementwise multiply with a
per-step coefficient row and a grouped reduction.

This walrus build cannot compile Softplus/Sigmoid/Tanh together (no common
ACT table set; Softplus crashes lower_act outright), so everything is built
from the `natural_log_exp_and_others` set only:

    softplus(a) = ln(exp(a) + 1)           (+1 folded into the ACT bias)
    sigmoid(a)  = 1 - R,  R = recip(exp(a)+1)      (DVE reciprocal)
    tanh(u)     = 1 - 2*recip(exp(2u)+1)

The affine parts of sigmoid/tanh are folded into pre-negated/scaled weight
copies (Wv0·(-2), -Wv1, sign-flipped Levy coefficients), so the JVP still
costs one gate multiply per layer.

Scan state h lives on partitions (64p = state dim); the scan is fully
unrolled (128 steps, ~35 instructions each).  Per-step coefficient rows
(Levy areas, increments) are computed on device with the 128 windows on
partitions, then replicated across the 64 state partitions via a DRAM
round-trip so the tail multiply can read them with a stride-0 AP.
"""

import sys
import numpy as np

sys.path.insert(0, "/opt/trn_rl_repo")

import concourse.bass as bass
import concourse.mybir as mybir
from concourse import tile
from concourse.bass_utils import run_bass_kernel_spmd

D = 8
S = 64
H = 128
OUT = 8
T = 2049
B = 32
WIN = 16
NWIN = (T - 1) // WIN  # 128
NC_CORES = 8
BC = B // NC_CORES  # 4 batch elements per core

F32 = mybir.dt.float32
AF = mybir.ActivationFunctionType
OP = mybir.AluOpType
X = mybir.AxisListType.X

_PATCHED = False


def _patch_tile_drain():
    """This walrus build rejects sem waits attached to Drain instructions
    ("Too many sync wait commands").  Emit the end-of-kernel waits as
    standalone wait_ge instructions instead."""
    global _PATCHED
    if _PATCHED:
        return
    _PATCHED = True

    def _drain_and_barrier(self, tick_clock, wait_clock):
        nc = self.nc
        carrier = nc.sync.nop()
        wait_clock.add_sem_waits(
            carrier.ins, tile.ScopedClock({None: tick_clock.global_clock})
        )
        ws = list(carrier.ins.sync_info.on_wait)
        carrier.ins.sync_info = mybir.SyncInfo(on_wait=[], on_update=[])
        by_name = {s.name: s for s in self.sems.allocated().values()}
        for w in ws:
            s = by_name.get(w.ant_name)
            if s is not None:
                nc.sync.wait_ge(s, w.wait_value)
        nc.sync.drain()
        nc.all_engine_barrier()
        popped = nc._tile_sem_poison_stack.pop()
        assert popped is self._sem_poison
        nc.clear_and_free_semaphores(list(self.sems.allocated().values()))
        nc.all_engine_barrier()

    tile.TileContext._drain_and_barrier = _drain_and_barrier


def _ap(base, offset, dims):
    return bass.AP(tensor=base.tensor, offset=offset, ap=[list(d) for d in dims])


WEIGHT_SPECS = [
    ("Wi0T", [D, H]), ("BI0", [1, H]),
    ("Wi1T", [H, H]), ("BI1", [1, H]),
    ("Wi2T", [H, S]), ("BI2", [1, S]),
    ("BV0", [1, H]),
    ("Wv1T", [H, H]), ("BV1", [1, H]),
    ("Wv2Tp", [H, D * S]),  # columns reordered to (k, dlo, a) channel pairs
    ("BV2P", [4, H]),       # bv2 in pair layout: [k', (dlo, a)]
    ("EYE4", [4, 4 * 4]),   # k' selector for the bias preload
    ("Wv0m2T", [H, H]),     # [ -2*Wv0^T ; 0 ] stacked   (even-channel JVP L0)
    ("Wv0m2B", [H, H]),     # [ 0 ; -2*Wv0^T ] stacked   (odd-channel JVP L0)
    ("RS0", [1, H]),        # row sums of Wv0 (Wv0 @ ones)
    ("ONES", [1, 512]),
    ("Wv1Tn", [H, H]),      # -Wv1^T       (gate sign fold)
    ("Wv0T2x", [H, H]),     # [Wv0^T ; Wv0^T]  (split-state forward mm)
    ("WrT2x", [H, OUT]), ("BR", [1, OUT]),
]



def _hoist_excess_waits(nc, max_waits=1):
    """walrus (this build) supports at most one sync-wait on most
    instructions and none on Drain; hoist the extras into standalone
    EventSemaphore instructions just before the owner, same engine."""
    cnt = 0
    for f in nc.m.functions:
        for bb in f.blocks:
            insts = list(bb.instructions)
            out = []
            changed = False
            for ins in insts:
                si = ins.sync_info
                ws = list(si.on_wait) if si is not None else []
                limit = 0 if type(ins).__name__ == "InstDrain" else max_waits
                if len(ws) > limit:
                    keep, extra = ws[:limit], ws[limit:]
                    for wi in extra:
                        cnt += 1
                        ev = mybir.InstEventSemaphore(name=f"HOISTW-{cnt}", ins=[], outs=[])
                        ev.engine = ins.engine
                        ev.sync_info = mybir.SyncInfo(on_wait=[wi], on_update=[])
                        out.append(ev)
                    ins.sync_info = mybir.SyncInfo(on_wait=keep, on_update=list(si.on_update))
                    changed = True
                out.append(ins)
            if changed:
                bb.instructions = out
    return cnt


def build_nc(scan_loop_reps=0, variant="full", n_steps=NWIN, hoist=True):
    _patch_tile_drain()
    nc = bass.Bass()

    xd = nc.dram_tensor("x", [BC, T, D], F32, kind="ExternalInput")
    wi = {}
    for name, shape in WEIGHT_SPECS:
        wi[name] = nc.dram_tensor(name, shape, F32, kind="ExternalInput")
    outd = nc.dram_tensor("out", [BC, NWIN + 1, OUT], F32, kind="ExternalOutput")

    with tile.TileContext(nc) as tc:
        with (
            tc.tile_pool(name="singles", bufs=1) as SG,
            tc.tile_pool(name="dram", bufs=1, space="DRAM") as DR,
        ):
            w = {}
            for name, shape in WEIGHT_SPECS:
                w[name] = SG.tile(list(shape), F32, tag=name, name=name)
                nc.sync.dma_start(out=w[name][:], in_=wi[name][:])

            hist = SG.tile([H, 4 * (NWIN + 1)], F32, tag="hist")  # split h: top+bottom halves sum to h
            s1full = SG.tile([H, NWIN * 16], F32, tag="s1full")  # (128, 2048)
            afull = [SG.tile([H, 32 * 128], F32, tag=f"af{c}", name=f"af{c}")
                     for c in range(4)]
            a_dram = DR.tile([NWIN, 256], F32)
            s1_dram = DR.tile([NWIN, 32], F32)
            atile = SG.tile([NWIN, 256], F32, tag="atile")
            s1sb = SG.tile([NWIN, 32], F32, tag="s1sb")

            if n_steps < NWIN:
                nc.vector.memset(hist[:], 0.0)  # sim-only partial-scan runs
            xbase = xd[:]

            # ---------------- phase A: log-signatures ----------------
            with (
                tc.tile_pool(name="pa", bufs=2) as PA,
                tc.tile_pool(name="pap", bufs=2, space="PSUM") as PAP,
            ):
                for b in range(BC):
                    xa = PA.tile([128, 128], F32, tag="xa")
                    nc.sync.dma_start(
                        out=xa[:], in_=_ap(xbase, b * T * D, [[128, 128], [1, 128]])
                    )
                    xw = PA.tile([128, 128], F32, tag="xw")
                    nc.sync.dma_start(
                        out=xw[:], in_=_ap(xbase, b * T * D + D, [[128, 128], [1, 128]])
                    )
                    # deltas, with 64 zero pad columns in front for the shifts
                    dlt = PA.tile([128, 192], F32, tag="dlt")
                    nc.vector.memset(dlt[:, 0:64], 0.0)
                    nc.vector.tensor_sub(dlt[:, 64:192], xw[:], xa[:])
                    dview = dlt[:, 64:192]
                    # s1[n, e] (scaled by -2 for the tail STT) -> s1sb col e*4+b
                    s1r = PA.tile([128, 8], F32, tag="s1r")
                    nc.vector.tensor_reduce(
                        out=s1r[:],
                        in_=dview.rearrange("p (w d) -> p d w", d=D),
                        axis=X,
                        op=OP.add,
                    )
                    s1b = s1r[:]
                    for dlo in range(2):
                        nc.vector.tensor_scalar(
                            out=_ap(s1sb[:], dlo * 16 + b, [[32, 128], [4, 4]]),
                            in0=_ap(s1b, s1b.offset + dlo, [list(s1b.ap[0]), [2, 4]]),
                            scalar1=-2.0, scalar2=None, op0=OP.mult,
                        )
                    # inclusive prefix sums over w (shifts of 1,2,4,8 windows)
                    cprev = dlt
                    for k, sh in enumerate((8, 16, 32, 64)):
                        cn = PA.tile([128, 192], F32, tag=f"c{k}")
                        nc.vector.memset(cn[:, 0:64], 0.0)
                        nc.vector.tensor_add(
                            cn[:, 64:192], cprev[:, 64:192], cprev[:, 64 - sh:192 - sh]
                        )
                        cprev = cn
                    ex = PA.tile([128, 128], F32, tag="ex")
                    nc.vector.tensor_sub(ex[:], cprev[:, 64:192], dlt[:, 64:192])
                    # M[n, i, j] = sum_w ex[w, i] * dlt[w, j]
                    prod = PA.tile([128, 1024], F32, tag="prod")
                    exb = ex[:]
                    nc.vector.tensor_mul(
                        prod[:],
                        _ap(exb, exb.offset, [list(exb.ap[0]), [1, 8], [0, 8], [8, 16]]),
                        _ap(dview, dview.offset,
                            [list(dview.ap[0]), [0, 8], [1, 8], [8, 16]]),
                    )
                    mred = PA.tile([128, 64], F32, tag="mred")
                    nc.vector.tensor_reduce(
                        out=mred[:],
                        in_=prod[:].rearrange("p (i j w) -> p i j w", i=8, j=8),
                        axis=X,
                        op=OP.add,
                    )
                    # coefficient A4[n, d, e] = 2*(M[e,d] - M[d,e]),
                    # scattered to cols dlo*128 + k*32 + par_e*16 + ke*4 + b
                    # (d = 2k+dlo rides the partition half of the U tiles,
                    #  e enumerated as par_e*4+ke = JVP col order)
                    ldf = PA.tile([128, 64], F32, tag="ldf")
                    mb = mred[:]
                    nc.vector.tensor_sub(
                        ldf[:],
                        _ap(mb, mb.offset, [list(mb.ap[0]), [1, 8], [8, 8]]),
                        _ap(mb, mb.offset, [list(mb.ap[0]), [8, 8], [1, 8]]),
                    )
                    lfb = ldf[:]
                    for dlo in range(2):
                        nc.vector.tensor_scalar(
                            out=_ap(atile[:], dlo * 128 + b,
                                    [[256, 128], [32, 4], [16, 2], [4, 4]]),
                            in0=_ap(lfb, lfb.offset + dlo * 8,
                                    [list(lfb.ap[0]), [16, 4], [1, 2], [2, 4]]),
                            scalar1=2.0, scalar2=None, op0=OP.mult,
                        )

                # a_dram layout: (dlo, n, 128) ; s1_dram: (dlo, n, 16)
                adb = a_dram[:]
                nc.sync.dma_start(
                    out=_ap(adb, 0, [[128, 128], [128 * 128, 2], [1, 128]]),
                    in_=atile[:].rearrange("p (l c) -> p l c", l=2),
                )
                s1db = s1_dram[:]
                nc.sync.dma_start(
                    out=_ap(s1db, 0, [[16, 128], [16 * 128, 2], [1, 16]]),
                    in_=s1sb[:].rearrange("p (l c) -> p l c", l=2),
                )
                # replicate: partitions 0:64 read the dlo=0 block, 64:128 dlo=1
                for c in range(4):
                    for dlo in range(2):
                        nc.gpsimd.dma_start(
                            out=afull[c][64 * dlo:64 * dlo + 64, :],
                            in_=_ap(adb, dlo * 128 * 128 + c * 32 * 128,
                                    [[0, S], [1, 32 * 128]]),
                        )
                for dlo in range(2):
                    nc.gpsimd.dma_start(
                        out=s1full[64 * dlo:64 * dlo + 64, :],
                        in_=_ap(s1db, dlo * 16 * 128, [[0, S], [1, NWIN * 16]]),
                    )

                # ---------------- initial MLP -> hist[:, 0:4] ----------------
                x0 = PA.tile([D, BC], F32, tag="x0")
                nc.sync.dma_start(out=x0[:], in_=_ap(xbase, 0, [[1, D], [T * D, BC]]))
                pi0 = PAP.tile([H, BC], F32, tag="pi")
                nc.tensor.matmul(pi0[:], w["BI0"][:], w["ONES"][:, 0:BC], start=True, stop=False)
                nc.tensor.matmul(pi0[:], w["Wi0T"][:], x0[:], start=False, stop=True)
                ei0 = PA.tile([H, BC], F32, tag="ei0")
                nc.scalar.activation(ei0[:], pi0[:], AF.Exp)
                zi0 = PA.tile([H, BC], F32, tag="zi0")
                nc.scalar.activation(zi0[:], ei0[:], AF.Ln, bias=1.0)
                pi1 = PAP.tile([H, BC], F32, tag="pi")
                nc.tensor.matmul(pi1[:], w["BI1"][:], w["ONES"][:, 0:BC], start=True, stop=False)
                nc.tensor.matmul(pi1[:], w["Wi1T"][:], zi0[:], start=False, stop=True)
                ei1 = PA.tile([H, BC], F32, tag="ei1")
                nc.scalar.activation(ei1[:], pi1[:], AF.Exp)
                zi1 = PA.tile([H, BC], F32, tag="zi1")
                nc.scalar.activation(zi1[:], ei1[:], AF.Ln, bias=1.0)
                ph = PAP.tile([S, BC], F32, tag="ph")
                nc.tensor.matmul(ph[:], w["BI2"][:], w["ONES"][:, 0:BC], start=True, stop=False)
                nc.tensor.matmul(ph[:], w["Wi2T"][:], zi1[:], start=False, stop=True)
                nc.scalar.activation(hist[0:S, 0:BC], ph[:], AF.Copy)
                nc.vector.memset(hist[S:H, 0:BC], 0.0)

            # ---------------- phase B: the scan ----------------
            import contextlib
            loop_ctx = (
                tc.For_i(0, scan_loop_reps, 1)
                if scan_loop_reps > 0 else contextlib.nullcontext()
            )
            with (
                tc.tile_pool(name="psa", bufs=2, space="PSUM") as PSA,
                tc.tile_pool(name="psu", bufs=2, space="PSUM") as PSU,
                tc.tile_pool(name="pst", bufs=2, space="PSUM") as PST,
                tc.tile_pool(name="psw", bufs=2, space="PSUM") as PSW,
                tc.tile_pool(name="step", bufs=2) as ST,
                loop_ctx,
            ):
                for n in range(n_steps):
                    hcur = hist[:, 4 * n:4 * n + 4]
                    # ---- forward MLP (softplus via exp/ln; h kept half-split) ----
                    a0 = PSA.tile([H, BC], F32, tag="psa")
                    nc.tensor.matmul(a0[:], w["BV0"][:], w["ONES"][:, 0:BC],
                                     start=True, stop=False)
                    nc.tensor.matmul(a0[:], w["Wv0T2x"][:], hcur,
                                     start=False, stop=True)
                    e0 = ST.tile([H, BC], F32, tag="e0")
                    nc.scalar.activation(e0[:], a0[:], AF.Exp)
                    z0 = ST.tile([H, BC], F32, tag="z0")
                    nc.scalar.activation(z0[:], e0[:], AF.Ln, bias=1.0)
                    a1 = PSA.tile([H, BC], F32, tag="psa")
                    nc.tensor.matmul(a1[:], w["BV1"][:], w["ONES"][:, 0:BC],
                                     start=True, stop=False)
                    nc.tensor.matmul(a1[:], w["Wv1T"][:], z0[:],
                                     start=False, stop=True)
                    e1 = ST.tile([H, BC], F32, tag="e1")
                    nc.scalar.activation(e1[:], a1[:], AF.Exp)
                    z1 = ST.tile([H, BC], F32, tag="z1")
                    nc.scalar.activation(z1[:], e1[:], AF.Ln, bias=1.0)
                    # ---- u layer, pair layout (128p=(dlo,a), cols k*4+b) ----
                    U = PSU.tile([H, 4 * BC], F32, tag="psu")
                    nc.tensor.matmul(U[:], w["BV2P"][:], w["EYE4"][:],
                                     start=True, stop=False)
                    for k in range(4):
                        nc.tensor.matmul(
                            U[:, 4 * k:4 * k + 4],
                            w["Wv2Tp"][:, H * k:H * k + H],
                            z1[:], start=False, stop=(k == 3),
                        )
                    UPD = ST.tile([H, 148], F32, tag="upd")
                    ub = UPD[:]
                    # h goes into the update tile early (off the critical path)
                    nc.vector.tensor_copy(
                        _ap(ub, ub.offset + 36, [list(ub.ap[0]), [37, 4]]), hcur
                    )
                    # Re = recip(1+exp(2u))
                    ee = ST.tile([H, 4 * BC], F32, tag="ee")
                    nc.scalar.activation(ee[:], U[:], AF.Exp, scale=2.0)
                    r0 = ST.tile([H, BC], F32, tag="r0")   # 1-sigmoid(a0)
                    nc.scalar.activation(r0[:], z0[:], AF.Exp, scale=-1.0)
                    r1 = ST.tile([H, BC], F32, tag="r1")
                    nc.scalar.activation(r1[:], z1[:], AF.Exp, scale=-1.0)
                    eep = ST.tile([H, 4 * BC], F32, tag="eep")
                    nc.vector.tensor_scalar(
                        out=eep[:], in0=ee[:], scalar1=1.0, scalar2=None, op0=OP.add
                    )
                    re = ST.tile([H, 4 * BC], F32, tag="re")
                    nc.vector.reciprocal(re[:], eep[:])
                    if variant == "nojvp":
                        nc.vector.scalar_tensor_tensor(
                            out=_ap(ub, ub.offset + 32, [list(ub.ap[0]), [1, 4], [37, 4]]),
                            in0=re[:].rearrange("p (k b) -> p k b", b=BC), scalar=0.5,
                            in1=s1full[:, 16 * n:16 * n + 16].rearrange("p (k b) -> p k b", b=BC),
                            op0=OP.subtract, op1=OP.mult,
                        )
                        nc.vector.tensor_reduce(
                            out=hist[:, 4 * (n + 1):4 * (n + 1) + 4],
                            in_=_ap(ub, ub.offset + 32, [list(ub.ap[0]), [37, 4], [1, 5]]),
                            axis=X, op=OP.add,
                        )
                        continue
                    # ssm = (Re-1)*Re (overlaps the t0 matmuls)
                    ssm = ST.tile([H, 4 * BC], F32, tag="ssm")
                    nc.vector.scalar_tensor_tensor(
                        out=ssm[:], in0=re[:], scalar=1.0, in1=re[:],
                        op0=OP.subtract, op1=OP.mult,
                    )
                    # ---- JVP ----
                    t0 = PST.tile([H, 8 * BC], F32, tag="pst")
                    nc.tensor.matmul(t0[:], w["RS0"][:], w["ONES"][:, 0:8 * BC],
                                     start=True, stop=False)
                    nc.tensor.matmul(t0[:, 0:16], w["Wv0m2T"][:], re[:],
                                     start=False, stop=False)
                    nc.tensor.matmul(t0[:, 16:32], w["Wv0m2B"][:], re[:],
                                     start=False, stop=True)
                    t0g = ST.tile([H, 8 * BC], F32, tag="t0g")  # = -t0*g0
                    r0b = r0[:]
                    nc.vector.scalar_tensor_tensor(
                        out=t0g[:],
                        in0=_ap(r0b, r0b.offset, [list(r0b.ap[0]), [0, 8], [1, 4]]),
                        scalar=1.0, in1=t0[:], op0=OP.subtract, op1=OP.mult,
                    )
                    # s1 part (fills the t1-matmul gap)
                    nc.vector.scalar_tensor_tensor(
                        out=_ap(ub, ub.offset + 32, [list(ub.ap[0]), [1, 4], [37, 4]]),
                        in0=re[:].rearrange("p (k b) -> p k b", b=BC), scalar=0.5,
                        in1=s1full[:, 16 * n:16 * n + 16].rearrange("p (k b) -> p k b", b=BC),
                        op0=OP.subtract, op1=OP.mult,
                    )
                    t1 = PST.tile([H, 8 * BC], F32, tag="pst")
                    nc.tensor.matmul(t1[:], w["Wv1Tn"][:], t0g[:], start=True, stop=True)
                    t1g = ST.tile([H, 8 * BC], F32, tag="t1g")  # = -t1*g1
                    r1b = r1[:]
                    nc.vector.scalar_tensor_tensor(
                        out=t1g[:],
                        in0=_ap(r1b, r1b.offset, [list(r1b.ap[0]), [0, 8], [1, 4]]),
                        scalar=1.0, in1=t1[:], op0=OP.subtract, op1=OP.mult,
                    )
                    # M2 = ssm ⊙ A4row (overlaps the W4 matmuls)
                    M2 = ST.tile([H, 128], F32, tag="m2")
                    ssb = ssm[:]
                    ac = afull[n // 32]
                    nc.vector.tensor_mul(
                        M2[:],
                        _ap(ssb, ssb.offset, [list(ssb.ap[0]), [4, 4], [0, 8], [1, 4]]),
                        ac[:, 128 * (n % 32):128 * (n % 32) + 128],
                    )
                    W4 = PSW.tile([H, 128], F32, tag="psw")  # = -W4, pair layout
                    for kw in range(4):
                        nc.tensor.matmul(
                            W4[:, 32 * kw:32 * kw + 32],
                            w["Wv2Tp"][:, H * kw:H * kw + H],
                            t1g[:], start=True, stop=True,
                        )
                    if variant == "notail":
                        nc.vector.tensor_reduce(
                            out=hist[:, 4 * (n + 1):4 * (n + 1) + 4],
                            in_=_ap(ub, ub.offset + 32, [list(ub.ap[0]), [37, 4], [1, 5]]),
                            axis=X, op=OP.add,
                        )
                        continue
                    w4b = W4[:]
                    m2b = M2[:]
                    nc.vector.tensor_mul(
                        _ap(ub, ub.offset, [list(ub.ap[0]), [37, 4], [8, 4], [1, 8]]),
                        _ap(w4b, w4b.offset, [list(w4b.ap[0]), [1, 4], [32, 4], [4, 8]]),
                        _ap(m2b, m2b.offset, [list(m2b.ap[0]), [1, 4], [32, 4], [4, 8]]),
                    )
                    # hist[n+1] = reduce over 4 contiguous 37-element runs (b-major)
                    nc.vector.tensor_reduce(
                        out=hist[:, 4 * (n + 1):4 * (n + 1) + 4],
                        in_=_ap(ub, ub.offset, [list(ub.ap[0]), [37, 4], [1, 37]]),
                        axis=X, op=OP.add,
                    )

            # ---------------- phase C: readout ----------------
            with (
                tc.tile_pool(name="psr", bufs=1, space="PSUM") as PSR,
                tc.tile_pool(name="ro", bufs=1) as RO,
            ):
                outsb = RO.tile([OUT, 4 * (NWIN + 1)], F32, tag="outsb")
                r0p = PSR.tile([OUT, 512], F32, tag="r0p")
                nc.tensor.matmul(r0p[:], w["BR"][:], w["ONES"][:], start=True, stop=False)
                nc.tensor.matmul(r0p[:], w["WrT2x"][:], hist[:, 0:512], start=False, stop=True)
                r1p = PSR.tile([OUT, 4], F32, tag="r1p")
                nc.tensor.matmul(r1p[:], w["BR"][:], w["ONES"][:, 0:4], start=True, stop=False)
                nc.tensor.matmul(r1p[:], w["WrT2x"][:], hist[:, 512:516], start=False, stop=True)
                nc.scalar.activation(outsb[:, 0:512], r0p[:], AF.Copy)
                nc.scalar.activation(outsb[:, 512:516], r1p[:], AF.Copy)
                ob = outsb[:]
                for b in range(BC):
                    nc.sync.dma_start(
                        out=_ap(outd[:], b * (NWIN + 1) * OUT, [[1, OUT], [OUT, NWIN + 1]]),
                        in_=_ap(ob, ob.offset + b, [list(ob.ap[0]), [4, NWIN + 1]]),
                    )
    if hoist:
        _hoist_excess_waits(nc)
    return nc


def host_inputs(ts, x, Wi0, bi0, Wi1, bi1, Wi2, bi2,
                Wv0, bv0, Wv1, bv1, Wv2, bv2, Wr, br):
    """Per-core input maps (weights replicated, batch sharded)."""
    f = lambda a: np.ascontiguousarray(np.asarray(a, dtype=np.float32))
    eye4 = np.zeros((4, 16), np.float32)
    for k in range(4):
        eye4[k, 4 * k:4 * k + 4] = 1.0
    # Wv2 rows regrouped into channel pairs: block k holds channels (2k, 2k+1)
    wv2 = np.asarray(Wv2, np.float32).reshape(D, S, H)
    wv2tp = np.ascontiguousarray(
        np.concatenate([wv2[2 * k:2 * k + 2].reshape(2 * S, H) for k in range(4)], axis=0).T
    )  # (H, 512) cols = k*128 + dlo*64 + a
    bv2p = np.ascontiguousarray(
        np.asarray(bv2, np.float32).reshape(D, S).reshape(4, 2 * S)
    )  # (4, 128) rows k' -> (dlo, a)
    shared = {
        "Wi0T": f(Wi0.T), "BI0": f(bi0).reshape(1, H),
        "Wi1T": f(Wi1.T), "BI1": f(bi1).reshape(1, H),
        "Wi2T": f(Wi2.T), "BI2": f(bi2).reshape(1, S),
        "BV0": f(bv0).reshape(1, H),
        "Wv1T": f(Wv1.T), "BV1": f(bv1).reshape(1, H),
        "Wv2Tp": wv2tp,
        "BV2P": bv2p,
        "EYE4": eye4,
        "Wv0m2T": np.vstack([f(-2.0 * Wv0.T), np.zeros((S, H), np.float32)]),
        "Wv0m2B": np.vstack([np.zeros((S, H), np.float32), f(-2.0 * Wv0.T)]),
        "RS0": f(Wv0.sum(axis=1)).reshape(1, H),
        "ONES": np.ones((1, 512), np.float32),
        "Wv1Tn": f(-Wv1.T),
        "Wv0T2x": np.vstack([f(Wv0.T), f(Wv0.T)]),
        "WrT2x": np.vstack([f(Wr.T), f(Wr.T)]), "BR": f(br).reshape(1, OUT),
    }
    x = f(x)
    in_maps = []
    for c in range(NC_CORES):
        m = dict(shared)
        m["x"] = np.ascontiguousarray(x[c * BC:(c + 1) * BC])
        in_maps.append(m)
    return in_maps


_CACHE = {}


def _make_runner(nc):
    """Persistent jitted PJRT runner (mirrors bass2jax.run_bass_via_pjrt's
    multi-core path, but keeps the jitted callable so repeat executions
    don't re-trace/re-compile)."""
    import jax
    from jax.experimental.shard_map import shard_map
    from jax.sharding import Mesh, PartitionSpec
    from concourse import bass2jax

    bass2jax.install_neuronx_cc_hook()
    partition_name = nc.partition_id_tensor.name if nc.partition_id_tensor else None
    in_names, out_names, out_avals, zero_shapes = [], [], [], []
    for alloc in nc.m.functions[0].allocations:
        if not isinstance(alloc, mybir.MemoryLocationSet):
            continue
        name = alloc.memorylocations[0].name
        if alloc.kind == "ExternalInput":
            if name != partition_name:
                in_names.append(name)
        elif alloc.kind == "ExternalOutput":
            out_names.append(name)
            shape = tuple(alloc.tensor_shape)
            dtype = mybir.dt.np(alloc.dtype)
            out_avals.append(jax.core.ShapedArray(shape, dtype))
            zero_shapes.append((shape, dtype))
    n_params, n_outs = len(in_names), len(out_names)
    all_in_names = tuple(in_names + out_names + ([partition_name] if partition_name else []))

    def _body(*args):
        operands = list(args)
        if partition_name:
            operands.append(bass2jax.partition_id_tensor())
        outs = bass2jax._bass_exec_p.bind(
            *operands,
            out_avals=tuple(out_avals),
            in_names=all_in_names,
            out_names=tuple(out_names),
            lowering_input_output_aliases=(),
            sim_require_finite=True,
            sim_require_nnan=True,
            nc=nc,
        )
        return tuple(outs)

    devices = jax.devices()[:NC_CORES]
    mesh = Mesh(np.asarray(devices), ("core",))
    sharded = jax.jit(
        shard_map(
            _body, mesh=mesh,
            in_specs=(PartitionSpec("core"),) * (n_params + n_outs),
            out_specs=(PartitionSpec("core"),) * n_outs,
            check_rep=False,
        ),
        donate_argnums=tuple(range(n_params, n_params + n_outs)),
        keep_unused=True,
    )

    def prep(in_maps):
        per_core = [[np.asarray(m[nm]) for nm in in_names] for m in in_maps]
        return [
            np.concatenate([per_core[c][i] for c in range(NC_CORES)], axis=0)
            for i in range(n_params)
        ]

    def run(concat_in):
        concat_zeros = [
            np.zeros((NC_CORES * s[0], *s[1:]), dt) for (s, dt) in zero_shapes
        ]
        out_arrs = sharded(*concat_in, *concat_zeros)
        jax.block_until_ready(out_arrs)
        return {out_names[i]: np.asarray(out_arrs[i]) for i in range(n_outs)}

    return prep, run


def _get_runner():
    if "runner" not in _CACHE:
        nc = build_nc()
        _CACHE["runner"] = _make_runner(nc)
    return _CACHE["runner"]


def kernel(**inputs) -> np.ndarray:
    in_maps = host_inputs(**inputs)
    prep, run = _get_runner()
    out = run(prep(in_maps))["out"]
    return np.ascontiguousarray(out.reshape(B, NWIN + 1, OUT).astype(np.float32))


if __name__ == "__main__":
    import os
    if not os.path.exists("/tmp/logncde_ref.npz"):
        import subprocess
        subprocess.run([sys.executable, "gen_expected.py"], check=True)
    dat = np.load("/tmp/logncde_ref.npz")
    inputs = {k: dat[k] for k in dat.files if k != "expected"}
    expected = dat["expected"]
    actual = kernel(**inputs)
    err = np.abs(actual - expected).max()
    print("max abs err:", err, "rel:", err / np.abs(expected).max())
